# revision 1
# baseline (speedup 1.0000x reference)
"""3-layer GAT on Trainium2, 8 NeuronCores.

Strategy (dst-sharded, slot-major, identity-matmul aggregation):
- Nodes dst-sharded across 8 cores (12500/core). Within a shard, dsts are
  permuted by in-degree so each block of 128 dsts has near-uniform degree.
- Edges of a block are laid out slot-major: slot j holds the j-th incoming
  edge of each of the 128 dsts (padded to the block max degree with a
  poisoned table row: h=0, a_src-logit=-60 => alpha ~ 0).
- Per-layer node feature tables T_l hold fp16 rows [h_l | as_l] so a single
  indirect row gather per block delivers both the features and the source
  attention logits. The dst logits come from a small resident per-shard
  table. Softmax is computed without the segment max (logits are O(1)).
- The scatter-add becomes PSUM accumulation of per-slot [128, F] tiles via
  matmuls with a stationary identity matrix.
- Layer outputs never leave the core: each block's epilogue (LN+ELU) feeds
  the next layer's table-row build (h_{l+1} = act @ W_{l+1}) on the spot.
  Full tables are assembled on the host between the 4 SPMD launches
  (collectives are not available in this runtime).
"""

import numpy as np

import concourse.bass as bass
import concourse.tile as tile
from concourse import bacc, mybir
from concourse.bass_utils import run_bass_kernel_spmd
from contextlib import ExitStack

F16 = mybir.dt.float16
F32 = mybir.dt.float32
I32 = mybir.dt.int32

CORES = 8
N = 100000
SH = 12500          # real dsts per core
SHP = 12544         # padded (98 * 128)
NB = SHP // 128     # 98 blocks
TBL_ROWS = CORES * SHP + 128   # + pad block
PAD_ROW = CORES * SHP
NEG = 0.2
EPS = 1e-5
PAD_AS = -60.0

# per-layer configs
L1 = dict(H=8, FT=128, TROW=136, ASOFF=128, TAILN=80, KW=128)
L2 = dict(H=4, FT=64, TROW=72, ASOFF=64, TAILN=32, KW=64)
L3 = dict(H=1, FT=16, TROW=16, ASOFF=10, TAILN=0, KW=0)


def _host_prep(edge_index):
    """Static index prep from edge_index only (graph layout, no NN compute)."""
    src = np.concatenate([edge_index[0].astype(np.int64), np.arange(N, dtype=np.int64)])
    dst = np.concatenate([edge_index[1].astype(np.int64), np.arange(N, dtype=np.int64)])
    deg = np.bincount(dst, minlength=N)

    row_of = np.empty(N, dtype=np.int64)
    order = np.argsort(dst, kind="stable")
    src_by_dst = src[order]
    starts = np.zeros(N + 1, dtype=np.int64)
    np.cumsum(deg, out=starts[1:])

    per_core = []
    for c in range(CORES):
        nodes = np.arange(c * SH, (c + 1) * SH)
        perm = np.argsort(deg[nodes], kind="stable")  # ascending degree
        perm_nodes = nodes[perm]
        npad = SHP - SH
        pos_node = np.full(SHP, -1, dtype=np.int64)
        pos_node[npad:] = perm_nodes
        row_of[perm_nodes] = c * SHP + npad + np.arange(SH)
        per_core.append(dict(pos_node=pos_node))

    # common per-block slot counts (max across cores)
    S_list = np.ones(NB, dtype=np.int64)
    for c in range(CORES):
        pos_node = per_core[c]["pos_node"]
        for b in range(NB):
            blk = pos_node[b * 128 : (b + 1) * 128]
            m = max((int(deg[n]) for n in blk if n >= 0), default=1)
            S_list[b] = max(S_list[b], max(1, m))
    offs = np.concatenate([[0], np.cumsum(S_list)]).astype(np.int64)
    totS = int(offs[-1])

    for c in range(CORES):
        pos_node = per_core[c]["pos_node"]
        idx_pm = np.full((128, totS), PAD_ROW, dtype=np.int32)
        for b in range(NB):
            o = offs[b]
            blk = pos_node[b * 128 : (b + 1) * 128]
            for p, n in enumerate(blk):
                if n < 0:
                    continue
                es = np.sort(src_by_dst[starts[n] : starts[n + 1]])
                idx_pm[p, o : o + len(es)] = row_of[es].astype(np.int32)
        per_core[c]["idx_pm"] = np.ascontiguousarray(idx_pm)
    return per_core, S_list, totS


def _wt_tables(inputs):
    W1, W2, W3 = (np.asarray(inputs["W1"], np.float32), np.asarray(inputs["W2"], np.float32),
                  np.asarray(inputs["W3"], np.float32))

    def fold(a):
        a = np.asarray(a, np.float32)
        H, F = a.shape
        m = np.zeros((H * F, H), np.float32)
        for k in range(H):
            m[k * F : (k + 1) * F, k] = a[k]
        return m

    wt1 = np.concatenate([W1, W1 @ fold(inputs["a_src1"]), W1 @ fold(inputs["a_dst1"])], axis=1)
    wt2 = np.concatenate(
        [W2, W2 @ fold(inputs["a_src2"]), np.zeros((128, 4), np.float32),
         W2 @ fold(inputs["a_dst2"]), np.zeros((128, 4), np.float32)], axis=1)
    wt3 = np.concatenate(
        [W3, W3 @ fold(inputs["a_src3"]), np.zeros((64, 5), np.float32),
         W3 @ fold(inputs["a_dst3"]), np.zeros((64, 15), np.float32)], axis=1)
    return wt1.astype(np.float16), wt2.astype(np.float16), wt3.astype(np.float16)


def _build_launch0():
    nc = bacc.Bacc("TRN2", target_bir_lowering=False, debug=False, num_devices=CORES)
    x_t = nc.dram_tensor("x_t", [128, SHP], F16, kind="ExternalInput")
    wt1 = nc.dram_tensor("wt1", [128, 144], F16, kind="ExternalInput")
    tb = nc.dram_tensor("tb", [SHP, 144], F16, kind="ExternalOutput")
    with tile.TileContext(nc) as tc, ExitStack() as ctx:
        cpool = ctx.enter_context(tc.tile_pool(name="c", bufs=1))
        epool = ctx.enter_context(tc.tile_pool(name="e", bufs=3))
        pp = ctx.enter_context(tc.tile_pool(name="pp", bufs=2, space="PSUM"))
        xs = cpool.tile([128, SHP], F16)
        nc.sync.dma_start(xs[:, :], x_t[:, :])
        ws = cpool.tile([128, 144], F16)
        nc.sync.dma_start(ws[:, :], wt1[:, :])
        for b in range(NB):
            ps = pp.tile([128, 144], F32, tag="ps")
            nc.tensor.matmul(ps[:, :], xs[:, b * 128 : (b + 1) * 128], ws[:, :],
                             start=True, stop=True)
            ev = epool.tile([128, 144], F16, tag="ev")
            nc.vector.tensor_copy(ev[:, :], ps[:, :])
            nc.sync.dma_start(tb[b * 128 : (b + 1) * 128, :], ev[:, :])
    nc.compile()
    return nc


def _build_agg(cfg, S_list, layer):
    TROW, H, FT, ASOFF, TAILN, KW = (cfg["TROW"], cfg["H"], cfg["FT"], cfg["ASOFF"],
                                     cfg["TAILN"], cfg["KW"])
    totS = int(np.sum(S_list))
    Smax = int(np.max(S_list))

    nc = bacc.Bacc("TRN2", target_bir_lowering=False, debug=False, num_devices=CORES)
    T = nc.dram_tensor("T", [TBL_ROWS, TROW], F16, kind="ExternalInput")
    idx_d = nc.dram_tensor("idx", [128, totS], I32, kind="ExternalInput")
    ad_d = nc.dram_tensor("ad", [128, NB * H], F16, kind="ExternalInput")
    ident_d = nc.dram_tensor("ident", [128, 128], F16, kind="ExternalInput")
    if layer < 3:
        wt_d = nc.dram_tensor("wt", [KW, TAILN], F16, kind="ExternalInput")
        out_d = nc.dram_tensor("out", [SHP, TAILN], F16, kind="ExternalOutput")
    else:
        out_d = nc.dram_tensor("out", [SHP, 10], F32, kind="ExternalOutput")

    with tile.TileContext(nc) as tc, ExitStack() as ctx:
        cpool = ctx.enter_context(tc.tile_pool(name="c", bufs=1))
        gpool = ctx.enter_context(tc.tile_pool(name="g", bufs=3))
        apool = ctx.enter_context(tc.tile_pool(name="a", bufs=3))
        epool = ctx.enter_context(tc.tile_pool(name="e", bufs=3))
        ppo = ctx.enter_context(tc.tile_pool(name="ppo", bufs=2, space="PSUM"))
        ppt = ctx.enter_context(tc.tile_pool(name="ppt", bufs=2, space="PSUM"))

        idx_sb = cpool.tile([128, totS], I32)
        nc.sync.dma_start(idx_sb[:, :], idx_d[:, :])
        ad_sb = cpool.tile([128, NB * H], F16)
        nc.sync.dma_start(ad_sb[:, :], ad_d[:, :])
        ident = cpool.tile([128, 128], F16)
        nc.sync.dma_start(ident[:, :], ident_d[:, :])
        if layer < 3:
            wt_sb = cpool.tile([KW, TAILN], F16)
            nc.sync.dma_start(wt_sb[:, :], wt_d[:, :])
        else:
            ls_sb = cpool.tile([128, NB, 16], F32)

        off = 0
        for b in range(NB):
            S = int(S_list[b])
            G = gpool.tile([128, Smax, TROW], F16, tag="G")
            # one gathered row per partition per instruction: multi-offset
            # indirect DMA is not supported by this runtime's DGE config
            for j in range(S):
                nc.gpsimd.indirect_dma_start(
                    out=G[:, j, :], out_offset=None, in_=T[:, :],
                    in_offset=bass.IndirectOffsetOnAxis(
                        ap=idx_sb[:, off + j : off + j + 1], axis=0))

            z = apool.tile([128, H, Smax], F32, tag="z")
            nc.vector.tensor_tensor(
                out=z[:, :, :S],
                in0=G[:, :S, ASOFF : ASOFF + H].rearrange("p s h -> p h s"),
                in1=ad_sb[:, b * H : (b + 1) * H].to_broadcast([128, H, S]),
                op=mybir.AluOpType.add)
            # exp(leaky_relu(z)) == max(exp(z), exp(NEG*z))
            e_a = apool.tile([128, H, Smax], F16, tag="e_a")
            nc.scalar.activation(e_a[:, :, :S], z[:, :, :S],
                                 mybir.ActivationFunctionType.Exp)
            e_b = apool.tile([128, H, Smax], F16, tag="e_b")
            nc.scalar.activation(e_b[:, :, :S], z[:, :, :S],
                                 mybir.ActivationFunctionType.Exp, scale=NEG)
            al = apool.tile([128, H, Smax], F16, tag="al")
            nc.vector.tensor_tensor(out=al[:, :, :S], in0=e_a[:, :, :S],
                                    in1=e_b[:, :, :S], op=mybir.AluOpType.max)
            s_t = apool.tile([128, H], F32, tag="s")
            nc.vector.tensor_reduce(s_t[:, :], al[:, :, :S], axis=mybir.AxisListType.X,
                                    op=mybir.AluOpType.add)
            rcp = apool.tile([128, H], F32, tag="rcp")
            nc.vector.reciprocal(rcp[:, :], s_t[:, :])
            ah = apool.tile([128, H, Smax], F16, tag="ah")
            nc.vector.tensor_tensor(out=ah[:, :, :S], in0=al[:, :, :S],
                                    in1=rcp[:, :].to_broadcast([128, H, S]),
                                    op=mybir.AluOpType.mult)
            ax = apool.tile([128, Smax, FT], F16, tag="ax")
            nc.scalar.activation(
                ax[:, :S, :].rearrange("p s (h r) -> p s h r", h=H),
                ah[:, :, :S].rearrange("p h s -> p s h").to_broadcast([128, S, H, FT // H]),
                mybir.ActivationFunctionType.Copy)
            Gp = gpool.tile([128, Smax, FT], F16, tag="Gp")
            nc.vector.tensor_tensor(out=Gp[:, :S, :], in0=G[:, :S, 0:FT], in1=ax[:, :S, :],
                                    op=mybir.AluOpType.mult)
            po = ppo.tile([128, FT], F32, tag="po")
            for j in range(S):
                nc.tensor.matmul(po[:, :], ident[:, :], Gp[:, j, :],
                                 start=(j == 0), stop=(j == S - 1))

            if layer < 3:
                bn6 = epool.tile([128, 6], F32, tag="bn6")
                nc.vector.bn_stats(bn6[:, :], po[:, :])
                mv = epool.tile([128, 2], F32, tag="mv")
                nc.vector.bn_aggr(mv[:, :], bn6[:, :])
                vpe = epool.tile([128, 1], F32, tag="vpe")
                nc.vector.tensor_scalar(out=vpe[:, :], in0=mv[:, 1:2], scalar1=EPS,
                                        scalar2=None, op0=mybir.AluOpType.add)
                sd = epool.tile([128, 1], F32, tag="sd")
                nc.scalar.activation(sd[:, :], vpe[:, :],
                                     mybir.ActivationFunctionType.Sqrt)
                rstd = epool.tile([128, 1], F32, tag="rstd")
                nc.vector.reciprocal(rstd[:, :], sd[:, :])
                y = epool.tile([128, FT], F32, tag="y")
                nc.vector.tensor_scalar(out=y[:, :], in0=po[:, :], scalar1=mv[:, 0:1],
                                        scalar2=rstd[:, :], op0=mybir.AluOpType.subtract,
                                        op1=mybir.AluOpType.mult)
                ym = epool.tile([128, FT], F32, tag="ym")
                nc.vector.tensor_scalar(out=ym[:, :], in0=y[:, :], scalar1=0.0,
                                        scalar2=None, op0=mybir.AluOpType.min)
                ee = epool.tile([128, FT], F32, tag="ee")
                nc.scalar.activation(ee[:, :], ym[:, :], mybir.ActivationFunctionType.Exp)
                e1 = epool.tile([128, FT], F32, tag="e1")
                nc.vector.tensor_scalar(out=e1[:, :], in0=ee[:, :], scalar1=1.0,
                                        scalar2=None, op0=mybir.AluOpType.subtract)
                act = epool.tile([128, FT], F16, tag="act")
                nc.vector.tensor_tensor(out=act[:, :], in0=y[:, :], in1=e1[:, :],
                                        op=mybir.AluOpType.max)
                pt = ppt.tile([FT, 128], F16, tag="pt")
                nc.tensor.transpose(pt[:, :], act[:, :], ident[:, :])
                at = epool.tile([FT, 128], F16, tag="at")
                nc.vector.tensor_copy(at[:, :], pt[:, :])
                p2 = ppt.tile([128, TAILN], F32, tag="p2")
                nc.tensor.matmul(p2[:, :], at[:, :], wt_sb[:, :], start=True, stop=True)
                ev = epool.tile([128, TAILN], F16, tag="ev")
                nc.vector.tensor_copy(ev[:, :], p2[:, :])
                nc.sync.dma_start(out_d[b * 128 : (b + 1) * 128, :], ev[:, :])
            else:
                nc.vector.tensor_copy(ls_sb[:, b, 0:10], po[:, 0:10])
            off += S

        if layer == 3:
            rmax = cpool.tile([128, NB], F32)
            nc.vector.tensor_reduce(rmax[:, :], ls_sb[:, :, 0:10], axis=mybir.AxisListType.X,
                                    op=mybir.AluOpType.max)
            tt = cpool.tile([128, NB, 10], F32)
            nc.vector.tensor_tensor(out=tt[:, :, :], in0=ls_sb[:, :, 0:10],
                                    in1=rmax[:, :].to_broadcast([128, NB, 10]),
                                    op=mybir.AluOpType.subtract)
            ex = cpool.tile([128, NB, 10], F32)
            nc.scalar.activation(ex[:, :, :], tt[:, :, :], mybir.ActivationFunctionType.Exp)
            ssum = cpool.tile([128, NB], F32)
            nc.vector.tensor_reduce(ssum[:, :], ex[:, :, :], axis=mybir.AxisListType.X,
                                    op=mybir.AluOpType.add)
            lg = cpool.tile([128, NB], F32)
            nc.scalar.activation(lg[:, :], ssum[:, :], mybir.ActivationFunctionType.Ln)
            fin = cpool.tile([128, NB, 10], F32)
            nc.vector.tensor_tensor(out=fin[:, :, :], in0=tt[:, :, :],
                                    in1=lg[:, :].to_broadcast([128, NB, 10]),
                                    op=mybir.AluOpType.subtract)
            nc.sync.dma_start(out_d.ap().rearrange("(b p) c -> p b c", p=128), fin[:, :, :])
    nc.compile()
    return nc


def _ad_layout(tbl, H):
    """[SHP, H] (perm order) -> [128, NB*H] with [p, b*H+k] = tbl[b*128+p, k]."""
    return np.ascontiguousarray(tbl.reshape(NB, 128, H).transpose(1, 0, 2).reshape(128, NB * H))


LAST_EXEC_NS = 0


def _hw_runner(nc, in_maps):
    global LAST_EXEC_NS
    try:
        r = run_bass_kernel_spmd(nc, in_maps, core_ids=list(range(CORES)), trace=True)
        if r.exec_time_ns:
            LAST_EXEC_NS += r.exec_time_ns
    except Exception:
        r = run_bass_kernel_spmd(nc, in_maps, core_ids=list(range(CORES)), trace=False)
    return r.results


def kernel(_runner=None, **inputs):
    runner = _runner or _hw_runner
    x = np.asarray(inputs["x"])
    edge_index = np.asarray(inputs["edge_index"])
    pc, S_list, totS = _host_prep(edge_index)
    wt1, wt2, wt3 = _wt_tables(inputs)
    ident = np.eye(128, dtype=np.float16)

    # ---- launch 0: T1 shard build ----
    nc0 = _build_launch0()
    in_maps0 = []
    for c in range(CORES):
        pos_node = pc[c]["pos_node"]
        xs = np.zeros((SHP, 128), np.float16)
        real = pos_node >= 0
        xs[real] = x[pos_node[real]].astype(np.float16)
        in_maps0.append({"x_t": np.ascontiguousarray(xs.T), "wt1": wt1})
    r0 = runner(nc0, in_maps0)
    tb = [r0[c]["tb"] for c in range(CORES)]

    T = np.zeros((TBL_ROWS, 136), np.float16)
    for c in range(CORES):
        T[c * SHP : (c + 1) * SHP] = tb[c][:, 0:136]
    T[PAD_ROW:, 128:136] = PAD_AS
    ads = [_ad_layout(tb[c][:, 136:144], 8) for c in range(CORES)]

    for layer, cfg in ((1, L1), (2, L2), (3, L3)):
        nc = _build_agg(cfg, S_list, layer)
        in_maps = []
        for c in range(CORES):
            m = {"T": T, "idx": pc[c]["idx_pm"], "ad": ads[c], "ident": ident}
            if layer == 1:
                m["wt"] = wt2
            elif layer == 2:
                m["wt"] = wt3
            in_maps.append(m)
        r = runner(nc, in_maps)
        outs = [r[c]["out"] for c in range(CORES)]
        if layer == 1:
            T = np.zeros((TBL_ROWS, 72), np.float16)
            for c in range(CORES):
                T[c * SHP : (c + 1) * SHP] = outs[c][:, 0:72]
            T[PAD_ROW:, 64:68] = PAD_AS
            ads = [_ad_layout(outs[c][:, 72:76], 4) for c in range(CORES)]
        elif layer == 2:
            T = np.zeros((TBL_ROWS, 16), np.float16)
            for c in range(CORES):
                T[c * SHP : (c + 1) * SHP] = outs[c][:, 0:16]
            T[PAD_ROW:, 10] = PAD_AS
            ads = [_ad_layout(outs[c][:, 16:17], 1) for c in range(CORES)]

    # un-permute final outputs
    result = np.empty((N, 10), np.float32)
    for c in range(CORES):
        pos_node = pc[c]["pos_node"]
        real = pos_node >= 0
        result[pos_node[real]] = outs[c][real]
    return result

